# revision 24
# baseline (speedup 1.0000x reference)
"""Trainium2 Bass kernel for nn_BuildVolume2dChaos (bilinear-warp cost volume).

kernel(refimg_fea, targetimg_fea, disps) -> volume [B=2, D=32, H=128, W=256]

Self-contained: builds an SPMD Bass program (one per-core variant), shards
inputs over 8 NeuronCores as (b, h-slice) = (core//4, 32*(core%4)), runs via
concourse.bass_utils.run_bass_kernel_spmd, reassembles the full output.

Algorithm per core (b fixed, 32 h-rows):
  vertical lerp of the target features (grid_sample align_corners=False row
  weights) -> Tv; horizontal bilinear warp expressed as a banded matmul:
  warped[c, (d,w)] = sum_{w'} Tv[c,w'] * relu(1 - |ix(d,w) - w'|) with
  ix = (w - disp)*W/(W-1) - 0.5 and zero-padded Tv. Five 62-wide w-tiles give
  a 128-row w'-window each -> K=128, M=32 matmuls on the PE, one d-octant at
  a time, packed 4-up in PSUM via tile_position. Tent weights are built by
  replicating ix across partitions with a stride-0 DMA from a pre-tiled DRAM
  scratch copy (one contiguous descriptor per partition), then a 2-op tent
  (Abs with per-partition bias, then clamped affine Relu) on ScalarE, in
  bf16. |ref - warped| reduces over channels with a sliding block-diagonal
  ones matmul accumulated per g-octet in PSUM.
"""
import sys

sys.path.insert(0, '/opt/trn_rl_repo')

import numpy as np
import bass_rust
import concourse.bass as bass
import concourse.mybir as mybir
from concourse.tile import TileContext
from concourse.vector_clock import ScopedClock

f32 = mybir.dt.float32
bf16 = mybir.dt.bfloat16
Alu = mybir.AluOpType
ActF = mybir.ActivationFunctionType

B, C, H, W, D = 2, 32, 128, 256, 32
HS = 32
NCORES = 8
BASES = [0, 62, 124, 186, 248]
SIZES = [62, 62, 62, 62, 8]
NT = len(BASES)
TOFF = [62 * k - 65 for k in range(NT)]
XSCALE = W / (W - 1)
WP = W + 64

_MAX_WAITS = 1


def _split_excess_waits(nc, max_waits=_MAX_WAITS):
    """Walrus (this neuronx-cc XLA path) rejects instructions carrying more
    than ~1 sem-wait ('Too many sync wait commands'). Hoist excess waits onto
    same-engine Drain instructions inserted immediately before."""
    n_fixed = 0
    for f in nc.m.functions:
        for bb in f.blocks:
            insts = bb.instructions
            i = 0
            while i < len(insts):
                ins = insts[i]
                si = ins.sync_info
                if si is not None and si.on_wait and len(si.on_wait) > max_waits:
                    waits = list(si.on_wait)
                    ins.sync_info = bass_rust.SyncInfo(
                        on_wait=waits[:max_waits], on_update=list(si.on_update))
                    pre = []
                    for jj in range(max_waits, len(waits), max_waits):
                        d = mybir.InstDrain(
                            name=f"{ins.name}-ws{jj}", ins=[], outs=[])
                        d.engine = ins.engine
                        d.sync_info = bass_rust.SyncInfo(
                            on_wait=waits[jj:jj + max_waits], on_update=[])
                        pre.append(d)
                    for d in reversed(pre):
                        insts.insert(i, d)
                        nc.register_instruction(d, overwrite=True)
                    i += len(pre)
                    n_fixed += 1
                i += 1
    return n_fixed


class _PatchedTileContext(TileContext):
    """Walrus CoreV3 rejects instructions with >1 sem-wait ('Too many sync
    wait commands'); split the kernel-tail drain's waits across drains."""

    def __exit__(self, exc_type, exc_val, exc_tb):
        ret = super().__exit__(exc_type, exc_val, exc_tb)
        if exc_type is None:
            _split_excess_waits(self.nc)
        return ret

    def _drain_and_barrier(self, tick_clock, wait_clock):
        nc = self.nc
        drain_inst = nc.sync.drain()
        wait_clock.add_sem_waits(
            drain_inst.ins, ScopedClock({None: tick_clock.global_clock})
        )
        si = drain_inst.ins.sync_info
        if si is not None and si.on_wait and len(si.on_wait) > _MAX_WAITS:
            waits = list(si.on_wait)
            drain_inst.ins.sync_info = bass_rust.SyncInfo(
                on_wait=waits[:_MAX_WAITS], on_update=list(si.on_update)
            )
            for i in range(_MAX_WAITS, len(waits), _MAX_WAITS):
                extra = nc.sync.drain()
                extra.ins.sync_info = bass_rust.SyncInfo(
                    on_wait=waits[i: i + _MAX_WAITS], on_update=[]
                )
        nc.all_engine_barrier()
        assert self.sems is not None
        popped = nc._tile_sem_poison_stack.pop()
        assert popped is self._sem_poison
        nc.clear_and_free_semaphores(list(self.sems.allocated().values()))
        nc.all_engine_barrier()


def build_nc(reps=1):
    nc = bass.Bass("TRN2", debug=False, enable_asserts=False)

    dispst = nc.dram_tensor("dispst", [HS, D, W], f32, kind="ExternalInput")
    wrow = nc.dram_tensor("wrow", [HS, W], f32, kind="ExternalInput")
    tga = [nc.dram_tensor(f"tga{k}", [128, HS, C], f32, kind="ExternalInput")
           for k in range(NT)]
    tgb = [nc.dram_tensor(f"tgb{k}", [128, HS, C], f32, kind="ExternalInput")
           for k in range(NT)]
    wyb = nc.dram_tensor("wyb", [128, HS], f32, kind="ExternalInput")
    refrep = nc.dram_tensor("refrep", [128, HS, W], f32, kind="ExternalInput")
    wpb = nc.dram_tensor("wpb", [128, NT], f32, kind="ExternalInput")
    bdp2 = nc.dram_tensor("bdp2", [128, 64], f32, kind="ExternalInput")
    ixs = nc.dram_tensor("ixs", [NT, HS, D, 64], f32,
                         kind="ExternalOutput")
    vol = nc.dram_tensor("vol", [D, HS, W], f32, kind="ExternalOutput")
    vol_v = vol.ap().rearrange("(dq dp) h w -> h dq dp w", dq=4, dp=8)

    GP = 2          # g-rows per tent chunk
    NK = 512        # j-cols per warp matmul (8 d x 64 w-slots)

    with _PatchedTileContext(nc) as tc:
        with (
            tc.tile_pool(name="const", bufs=1) as cpool,
            tc.tile_pool(name="tv", bufs=1) as tvpool,
            tc.tile_pool(name="ixr", bufs=2) as xrpool,
            tc.tile_pool(name="tent", bufs=2) as tpool,
            tc.tile_pool(name="work", bufs=3) as wpool,
            tc.tile_pool(name="yvp", bufs=2) as yvpool,
            tc.tile_pool(name="outs", bufs=2) as opool,
            tc.tile_pool(name="pw", bufs=3, space="PSUM") as pw_pool,
            tc.tile_pool(name="po", bufs=2, space="PSUM") as po_pool,
        ):
            s_wrow = cpool.tile([HS, W], f32, tag="wrow")
            nc.sync.dma_start(s_wrow[:, :], wrow[:, :])
            s_wyb = cpool.tile([128, HS], f32, tag="wyb")
            nc.sync.dma_start(s_wyb[:, :], wyb[:, :])
            s_ref = cpool.tile([128, HS, W + 64], bf16, tag="ref")
            nc.vector.memset(s_ref[:, :, :], 0.0)
            with tc.tile_pool(name="refstg", bufs=1) as rpool:
                for rq in range(4):
                    s_reff = rpool.tile([128, 8, W], f32, tag="reff")
                    nc.sync.dma_start(s_reff[:, :, :],
                                      refrep[:, 8 * rq:8 * (rq + 1), :])
                    nc.vector.tensor_copy(s_ref[:, 8 * rq:8 * (rq + 1), :W],
                                          s_reff[:, :, :])
            s_wpb = cpool.tile([128, NT], f32, tag="wpb")
            nc.sync.dma_start(s_wpb[:, :], wpb[:, :])
            s_bdp = cpool.tile([128, 64], f32, tag="bdp")
            nc.sync.dma_start(s_bdp[:, :], bdp2[:, :])
            s_ix = cpool.tile([HS, D, WP], f32, tag="ix")
            nc.vector.memset(s_ix[:, :, :], 1.0e6)
            s_tv = []
            for k in range(NT):
                s_tv.append(tvpool.tile([128, HS, C], bf16, tag=f"tv{k}",
                                        name=f"tv{k}"))

            with tc.tile_pool(name="ixp", bufs=1) as xpool:
                DH = D // 2
                wrow_b = s_wrow[:, :].unsqueeze(1).broadcast_to([HS, DH, W])
                for half in range(2):
                    s_disp = xpool.tile([HS, DH * W], f32, tag="disp")
                    nc.sync.dma_start(
                        s_disp[:, :],
                        dispst[:, half * DH:(half + 1) * DH, :].rearrange(
                            "h d w -> h (d w)"))
                    nc.vector.scalar_tensor_tensor(
                        s_ix[:, half * DH:(half + 1) * DH, :W],
                        s_disp[:, :].rearrange("h (d w) -> h d w", d=DH),
                        -XSCALE, wrow_b, Alu.mult, Alu.add)
            # park ix in DRAM, pre-tiled per w-tile, so tent chunks can
            # replicate it across partitions with one contiguous descriptor
            # per partition
            for k in range(NT):
                nc.sync.dma_start(ixs.ap()[k],
                                  s_ix[:, :, BASES[k]:BASES[k] + 64])

            with tc.tile_pool(name="lerp", bufs=1) as lpool:
                wyb_b = s_wyb[:, :].unsqueeze(2).broadcast_to([128, HS, C])
                for k in range(NT):
                    ta = lpool.tile([128, HS, C], f32, tag="ta")
                    tb = lpool.tile([128, HS, C], f32, tag="tb")
                    nc.sync.dma_start(ta[:, :, :], tga[k][:, :, :])
                    nc.sync.dma_start(tb[:, :, :], tgb[k][:, :, :])
                    u = lpool.tile([128, HS, C], f32, tag="u")
                    nc.vector.tensor_tensor(u[:, :, :], tb[:, :, :],
                                            ta[:, :, :], Alu.subtract)
                    v = lpool.tile([128, HS, C], f32, tag="v")
                    nc.vector.tensor_tensor(v[:, :, :], u[:, :, :], wyb_b,
                                            Alu.mult)
                    nc.vector.tensor_tensor(s_tv[k][:, :, :], ta[:, :, :],
                                            v[:, :, :], Alu.add)

            for rep in range(reps):
                for k in range(NT):
                    T = SIZES[k]
                    base = BASES[k]
                    outp = None
                    stage = opool.tile([128, NK], f32, tag="stage")
                    WD = 8 if k == NT - 1 else 64
                    NKk = 8 * WD
                    for gp in range(HS // GP):
                        g0 = gp * GP
                        tent = tpool.tile([128, GP, 4, NK], bf16,
                                          tag="tent", name="tent")
                        ixr = xrpool.tile([128, GP, D, 64], f32,
                                          tag="ixr", name="ixr")
                        yv = yvpool.tile([128, GP * D * 64], f32,
                                         tag="yv", name="yv")
                        src = ixs.ap()[None, k, g0:g0 + GP,
                                       :, :].broadcast_to([128, GP, D, 64])
                        eng = nc.gpsimd if gp % 2 == 0 else nc.sync
                        eng.dma_start(ixr[:, :, :, :], src)
                        nc.scalar.activation(
                            yv[:, :],
                            ixr[:, :, :, :].rearrange("p a b c -> p (a b c)"),
                            ActF.Abs, bias=s_wpb[:, k:k + 1], scale=1.0)
                        nc.scalar.activation(
                            tent[:, :, :, :].rearrange(
                                "p a b c -> p (a b c)"),
                            yv[:, :], ActF.Relu, bias=1.0, scale=-1.0)
                        warped = pw_pool.tile([128, GP, NK], f32,
                                              tag="warped", name="warped")
                        for gi in range(GP):
                            for dq in range(4):
                                nc.tensor.matmul(
                                    warped[32 * dq:32 * (dq + 1), gi,
                                           0:NKk],
                                    s_tv[k][:, g0 + gi, :],
                                    tent[:, gi, dq, :].rearrange(
                                        "p (d w) -> p d w",
                                        w=64)[:, :, 0:WD],
                                    start=True, stop=True,
                                    tile_position=(0, 32 * dq))
                        df = wpool.tile([128, GP, NK], f32,
                                        tag="df", name="df")
                        for gi in range(GP):
                            ref_b = s_ref[:, g0 + gi:g0 + gi + 1,
                                          base:base + WD].broadcast_to(
                                [128, 8, WD])
                            nc.vector.scalar_tensor_tensor(
                                df[:, gi, 0:NKk].rearrange(
                                    "p (a b) -> p a b", a=8),
                                warped[:, gi, 0:NKk].rearrange(
                                    "p (a b) -> p a b", a=8),
                                -1.0, ref_b, Alu.mult, Alu.add)
                        adf = wpool.tile([128, GP, NK], f32,
                                         tag="adf", name="adf")
                        nc.vector.scalar_tensor_tensor(
                            adf[:, :, 0:NKk], df[:, :, 0:NKk], -1.0,
                            df[:, :, 0:NKk], Alu.mult, Alu.max)
                        for gi in range(GP):
                            gg = (g0 + gi) % 8
                            if gg == 0:
                                outp = po_pool.tile([32, NK], f32,
                                                    tag="outp",
                                                    name="outp")
                            nc.tensor.matmul(
                                outp[:, 0:NKk],
                                s_bdp[:, 32 - 4 * gg:64 - 4 * gg],
                                adf[:, gi, 0:NKk], start=(gg == 0),
                                stop=(gg == 7))
                            if gg == 7:
                                oct_ = (g0 + gi) // 8
                                if k == NT - 1:
                                    nc.vector.tensor_copy(
                                        stage[32 * oct_:32 * (oct_ + 1),
                                              :].rearrange(
                                            "p (dp w) -> p dp w",
                                            dp=8)[:, :, :WD],
                                        outp[:, 0:NKk].rearrange(
                                            "p (dp w) -> p dp w", w=WD))
                                else:
                                    nc.vector.tensor_copy(
                                        stage[32 * oct_:32 * (oct_ + 1), :],
                                        outp[:, :])
                    nc.sync.dma_start(
                        vol_v[:, :, :, base:base + T],
                        stage[:, :].rearrange(
                            "p (dp w) -> p dp w", dp=8)[:, :, :T])
    return nc


def _vertical_rows():
    h = np.arange(H)
    iy = h * (H / (H - 1)) - 0.5
    y0 = np.floor(iy).astype(int)
    wy1 = (iy - y0).astype(np.float32)
    return y0, wy1


def prep_core_inputs(refimg_fea, targetimg_fea, disps, core):
    b = core // 4
    h0 = HS * (core % 4)
    y0, wy1 = _vertical_rows()
    out = {}
    out["dispst"] = np.ascontiguousarray(
        disps[b, :, h0:h0 + HS, :].transpose(1, 0, 2)).astype(np.float32)
    w = np.arange(W, dtype=np.float32)
    out["wrow"] = np.broadcast_to(w * XSCALE - 0.5, (HS, W)).copy()
    tgt = targetimg_fea[b]
    for k in range(NT):
        wp = TOFF[k] + np.arange(128)
        wvalid = (wp >= 0) & (wp < W)
        ga = np.zeros((128, HS, C), np.float32)
        gb = np.zeros((128, HS, C), np.float32)
        tgt_t = np.ascontiguousarray(tgt.transpose(2, 1, 0))  # [W, H, C]
        gh = h0 + np.arange(HS)
        ra, rb = y0[gh], y0[gh] + 1
        rava = (ra >= 0) & (ra < H)
        rbva = (rb >= 0) & (rb < H)
        ga[np.ix_(wvalid, rava)] = tgt_t[wp[wvalid]][:, ra[rava], :]
        gb[np.ix_(wvalid, rbva)] = tgt_t[wp[wvalid]][:, rb[rbva], :]
        out[f"tga{k}"] = ga
        out[f"tgb{k}"] = gb
    out["wyb"] = np.broadcast_to(wy1[h0:h0 + HS], (128, HS)).copy()
    out["refrep"] = np.tile(refimg_fea[b, :, h0:h0 + HS, :],
                            (4, 1, 1)).astype(np.float32)
    p = np.arange(128, dtype=np.float32)
    out["wpb"] = np.stack([-(TOFF[k] + p) for k in range(NT)],
                          axis=1).astype(np.float32)
    bdpm = np.zeros((128, 64), np.float32)
    bdpm[np.arange(128), 32 + np.arange(128) // 32] = 1.0
    out["bdp2"] = bdpm
    return out


_NC_CACHE = {}


def _get_nc(reps=1):
    key = reps
    if key not in _NC_CACHE:
        _NC_CACHE[key] = build_nc(reps=reps)
    return _NC_CACHE[key]


def run(refimg_fea, targetimg_fea, disps, reps=1):
    from concourse.bass_utils import run_bass_kernel_spmd
    nc = _get_nc(reps=reps)
    in_maps = [prep_core_inputs(refimg_fea, targetimg_fea, disps, core)
               for core in range(NCORES)]
    res = run_bass_kernel_spmd(nc, in_maps, core_ids=list(range(NCORES)))
    full = np.empty((B, D, H, W), np.float32)
    for core in range(NCORES):
        b = core // 4
        h0 = HS * (core % 4)
        full[b, :, h0:h0 + HS, :] = res.results[core]["vol"]
    return full


def kernel(refimg_fea, targetimg_fea, disps):
    refimg_fea = np.asarray(refimg_fea, dtype=np.float32)
    targetimg_fea = np.asarray(targetimg_fea, dtype=np.float32)
    disps = np.asarray(disps, dtype=np.float32)
    return run(refimg_fea, targetimg_fea, disps)


# revision 26
# speedup vs baseline: 1.4185x; 1.4185x over previous
"""Trainium2 Bass kernel for nn_BuildVolume2dChaos (bilinear-warp cost volume).

kernel(refimg_fea, targetimg_fea, disps) -> volume [B=2, D=32, H=128, W=256]

Self-contained: builds an SPMD Bass program (one per-core variant), shards
inputs over 8 NeuronCores as (b, h-slice) = (core//4, 32*(core%4)), runs via
concourse.bass_utils.run_bass_kernel_spmd, reassembles the full output.

Algorithm per core (b fixed, 32 h-rows):
  vertical lerp of the target features (grid_sample align_corners=False row
  weights) -> Tv; horizontal bilinear warp expressed as a banded matmul:
  warped[c, (d,w)] = sum_{w'} Tv[c,w'] * relu(1 - |ix(d,w) - w'|) with
  ix = (w - disp)*W/(W-1) - 0.5 and zero-padded Tv. Five 62-wide w-tiles give
  a 128-row w'-window each -> K=128, M=32 matmuls on the PE, one d-octant at
  a time, packed 4-up in PSUM via tile_position. Tent weights are built by
  replicating ix across partitions with a stride-0 DMA from a pre-tiled DRAM
  scratch copy (one contiguous descriptor per partition), then a 2-op tent
  (Abs with per-partition bias, then clamped affine Relu) on ScalarE, in
  bf16. |ref - warped| reduces over channels with a sliding block-diagonal
  ones matmul accumulated per g-octet in PSUM.
"""
import sys

sys.path.insert(0, '/opt/trn_rl_repo')

import numpy as np
import bass_rust
import concourse.bass as bass
import concourse.mybir as mybir
from concourse.tile import TileContext
from concourse.vector_clock import ScopedClock

f32 = mybir.dt.float32
bf16 = mybir.dt.bfloat16
Alu = mybir.AluOpType
ActF = mybir.ActivationFunctionType

B, C, H, W, D = 2, 32, 128, 256, 32
HS = 32
NCORES = 8
BASES = [0, 62, 124, 186, 248]
SIZES = [62, 62, 62, 62, 8]
NT = len(BASES)
TOFF = [62 * k - 65 for k in range(NT)]
XSCALE = W / (W - 1)
WP = W + 64

_MAX_WAITS = 1


def _split_excess_waits(nc, max_waits=_MAX_WAITS):
    """Walrus (this neuronx-cc XLA path) rejects instructions carrying more
    than ~1 sem-wait ('Too many sync wait commands'). Hoist excess waits onto
    same-engine Drain instructions inserted immediately before."""
    n_fixed = 0
    for f in nc.m.functions:
        for bb in f.blocks:
            insts = bb.instructions
            i = 0
            while i < len(insts):
                ins = insts[i]
                si = ins.sync_info
                if si is not None and si.on_wait and len(si.on_wait) > max_waits:
                    waits = list(si.on_wait)
                    ins.sync_info = bass_rust.SyncInfo(
                        on_wait=waits[:max_waits], on_update=list(si.on_update))
                    pre = []
                    for jj in range(max_waits, len(waits), max_waits):
                        d = mybir.InstDrain(
                            name=f"{ins.name}-ws{jj}", ins=[], outs=[])
                        d.engine = ins.engine
                        d.sync_info = bass_rust.SyncInfo(
                            on_wait=waits[jj:jj + max_waits], on_update=[])
                        pre.append(d)
                    for d in reversed(pre):
                        insts.insert(i, d)
                        nc.register_instruction(d, overwrite=True)
                    i += len(pre)
                    n_fixed += 1
                i += 1
    return n_fixed


class _PatchedTileContext(TileContext):
    """Walrus CoreV3 rejects instructions with >1 sem-wait ('Too many sync
    wait commands'); split the kernel-tail drain's waits across drains."""

    def __exit__(self, exc_type, exc_val, exc_tb):
        ret = super().__exit__(exc_type, exc_val, exc_tb)
        if exc_type is None:
            _split_excess_waits(self.nc)
        return ret

    def _drain_and_barrier(self, tick_clock, wait_clock):
        nc = self.nc
        drain_inst = nc.sync.drain()
        wait_clock.add_sem_waits(
            drain_inst.ins, ScopedClock({None: tick_clock.global_clock})
        )
        si = drain_inst.ins.sync_info
        if si is not None and si.on_wait and len(si.on_wait) > _MAX_WAITS:
            waits = list(si.on_wait)
            drain_inst.ins.sync_info = bass_rust.SyncInfo(
                on_wait=waits[:_MAX_WAITS], on_update=list(si.on_update)
            )
            for i in range(_MAX_WAITS, len(waits), _MAX_WAITS):
                extra = nc.sync.drain()
                extra.ins.sync_info = bass_rust.SyncInfo(
                    on_wait=waits[i: i + _MAX_WAITS], on_update=[]
                )
        nc.all_engine_barrier()
        assert self.sems is not None
        popped = nc._tile_sem_poison_stack.pop()
        assert popped is self._sem_poison
        nc.clear_and_free_semaphores(list(self.sems.allocated().values()))
        nc.all_engine_barrier()


def build_nc(reps=1):
    nc = bass.Bass("TRN2", debug=False, enable_asserts=False)

    dispst = nc.dram_tensor("dispst", [HS, D, W], f32, kind="ExternalInput")
    wrow = nc.dram_tensor("wrow", [HS, W], f32, kind="ExternalInput")
    tga = [nc.dram_tensor(f"tga{k}", [128, HS, C], f32, kind="ExternalInput")
           for k in range(NT)]
    tgb = [nc.dram_tensor(f"tgb{k}", [128, HS, C], f32, kind="ExternalInput")
           for k in range(NT)]
    wyb = nc.dram_tensor("wyb", [128, HS], f32, kind="ExternalInput")
    refrep = nc.dram_tensor("refrep", [128, HS, W], f32, kind="ExternalInput")
    wpb = nc.dram_tensor("wpb", [128, NT], f32, kind="ExternalInput")
    bdp2 = nc.dram_tensor("bdp2", [128, 64], f32, kind="ExternalInput")
    ixs = nc.dram_tensor("ixs", [NT, HS, D, 64], f32,
                         kind="ExternalOutput")
    vol = nc.dram_tensor("vol", [D, HS, W], f32, kind="ExternalOutput")
    vol_v = vol.ap().rearrange("(dq dp) h w -> h dq dp w", dq=4, dp=8)

    GP = 2          # g-rows per tent chunk
    NK = 512        # j-cols per warp matmul (8 d x 64 w-slots)

    with _PatchedTileContext(nc) as tc:
        with (
            tc.tile_pool(name="const", bufs=1) as cpool,
            tc.tile_pool(name="tv", bufs=1) as tvpool,
            tc.tile_pool(name="ixr", bufs=2) as xrpool,
            tc.tile_pool(name="tent", bufs=2) as tpool,
            tc.tile_pool(name="work", bufs=3) as wpool,
            tc.tile_pool(name="yvp", bufs=2) as yvpool,
            tc.tile_pool(name="outs", bufs=2) as opool,
            tc.tile_pool(name="pw", bufs=3, space="PSUM") as pw_pool,
            tc.tile_pool(name="po", bufs=2, space="PSUM") as po_pool,
        ):
            s_wrow = cpool.tile([HS, W], f32, tag="wrow")
            nc.sync.dma_start(s_wrow[:, :], wrow[:, :])
            s_wyb = cpool.tile([128, HS], f32, tag="wyb")
            nc.sync.dma_start(s_wyb[:, :], wyb[:, :])
            s_ref = cpool.tile([128, HS, W + 64], bf16, tag="ref")
            nc.vector.memset(s_ref[:, :, :], 0.0)
            with tc.tile_pool(name="refstg", bufs=1) as rpool:
                for rq in range(4):
                    s_reff = rpool.tile([128, 8, W], f32, tag="reff")
                    nc.sync.dma_start(s_reff[:, :, :],
                                      refrep[:, 8 * rq:8 * (rq + 1), :])
                    nc.vector.tensor_copy(s_ref[:, 8 * rq:8 * (rq + 1), :W],
                                          s_reff[:, :, :])
            s_wpb = cpool.tile([128, NT], f32, tag="wpb")
            nc.sync.dma_start(s_wpb[:, :], wpb[:, :])
            s_bdp = cpool.tile([128, 64], f32, tag="bdp")
            nc.sync.dma_start(s_bdp[:, :], bdp2[:, :])
            s_ix = cpool.tile([HS, D, WP], f32, tag="ix")
            nc.vector.memset(s_ix[:, :, :], 1.0e6)
            s_tv = []
            for k in range(NT):
                s_tv.append(tvpool.tile([128, HS, C], bf16, tag=f"tv{k}",
                                        name=f"tv{k}"))

            with tc.tile_pool(name="ixp", bufs=1) as xpool:
                DH = D // 2
                wrow_b = s_wrow[:, :].unsqueeze(1).broadcast_to([HS, DH, W])
                for half in range(2):
                    s_disp = xpool.tile([HS, DH * W], f32, tag="disp")
                    nc.sync.dma_start(
                        s_disp[:, :],
                        dispst[:, half * DH:(half + 1) * DH, :].rearrange(
                            "h d w -> h (d w)"))
                    nc.vector.scalar_tensor_tensor(
                        s_ix[:, half * DH:(half + 1) * DH, :W],
                        s_disp[:, :].rearrange("h (d w) -> h d w", d=DH),
                        -XSCALE, wrow_b, Alu.mult, Alu.add)
            # park ix in DRAM, pre-tiled per w-tile, so tent chunks can
            # replicate it across partitions with one contiguous descriptor
            # per partition
            for k in range(NT):
                nc.sync.dma_start(ixs.ap()[k],
                                  s_ix[:, :, BASES[k]:BASES[k] + 64])

            with tc.tile_pool(name="lerp", bufs=1) as lpool:
                wyb_b = s_wyb[:, :].unsqueeze(2).broadcast_to([128, HS, C])
                for k in range(NT):
                    ta = lpool.tile([128, HS, C], f32, tag="ta")
                    tb = lpool.tile([128, HS, C], f32, tag="tb")
                    nc.sync.dma_start(ta[:, :, :], tga[k][:, :, :])
                    nc.sync.dma_start(tb[:, :, :], tgb[k][:, :, :])
                    u = lpool.tile([128, HS, C], f32, tag="u")
                    nc.vector.tensor_tensor(u[:, :, :], tb[:, :, :],
                                            ta[:, :, :], Alu.subtract)
                    v = lpool.tile([128, HS, C], f32, tag="v")
                    nc.vector.tensor_tensor(v[:, :, :], u[:, :, :], wyb_b,
                                            Alu.mult)
                    nc.vector.tensor_tensor(s_tv[k][:, :, :], ta[:, :, :],
                                            v[:, :, :], Alu.add)

            for rep in range(reps):
                for k in range(NT):
                    T = SIZES[k]
                    base = BASES[k]
                    outp = None
                    stage = opool.tile([128, NK], f32, tag="stage")
                    for gp in range(HS // GP):
                        g0 = gp * GP
                        tent = tpool.tile([128, GP, 4, NK], bf16,
                                          tag="tent", name="tent")
                        ixr = xrpool.tile([128, GP, D, 64], f32,
                                          tag="ixr", name="ixr")
                        yv = yvpool.tile([128, GP * D * 64], f32,
                                         tag="yv", name="yv")
                        src = ixs.ap()[None, k, g0:g0 + GP,
                                       :, :].broadcast_to([128, GP, D, 64])
                        nc.sync.dma_start(ixr[:, :, :, :], src)
                        nc.scalar.activation(
                            yv[:, :],
                            ixr[:, :, :, :].rearrange("p a b c -> p (a b c)"),
                            ActF.Abs, bias=s_wpb[:, k:k + 1], scale=1.0)
                        nc.scalar.activation(
                            tent[:, :, :, :].rearrange(
                                "p a b c -> p (a b c)"),
                            yv[:, :], ActF.Relu, bias=1.0, scale=-1.0)
                        warped = pw_pool.tile([128, GP, NK], f32,
                                              tag="warped", name="warped")
                        for gi in range(GP):
                            for dq in range(4):
                                nc.tensor.matmul(
                                    warped[32 * dq:32 * (dq + 1), gi, :],
                                    s_tv[k][:, g0 + gi, :],
                                    tent[:, gi, dq, :],
                                    start=True, stop=True,
                                    tile_position=(0, 32 * dq))
                        df = wpool.tile([128, GP, NK], f32,
                                        tag="df", name="df")
                        for gi in range(GP):
                            ref_b = s_ref[:, g0 + gi:g0 + gi + 1,
                                          base:base + 64].broadcast_to(
                                [128, 8, 64])
                            nc.vector.scalar_tensor_tensor(
                                df[:, gi, :].rearrange(
                                    "p (a b) -> p a b", a=8),
                                warped[:, gi, :].rearrange(
                                    "p (a b) -> p a b", a=8),
                                -1.0, ref_b, Alu.mult, Alu.add)
                        adf = wpool.tile([128, GP, NK], f32,
                                         tag="adf", name="adf")
                        nc.vector.scalar_tensor_tensor(
                            adf[:, :, :].rearrange("p g f -> p (g f)"),
                            df[:, :, :].rearrange("p g f -> p (g f)"),
                            -1.0,
                            df[:, :, :].rearrange("p g f -> p (g f)"),
                            Alu.mult, Alu.max)
                        for gi in range(GP):
                            gg = (g0 + gi) % 8
                            if gg == 0:
                                outp = po_pool.tile([32, NK], f32,
                                                    tag="outp",
                                                    name="outp")
                            nc.tensor.matmul(
                                outp[:, :],
                                s_bdp[:, 32 - 4 * gg:64 - 4 * gg],
                                adf[:, gi, :], start=(gg == 0),
                                stop=(gg == 7))
                            if gg == 7:
                                oct_ = (g0 + gi) // 8
                                nc.vector.tensor_copy(
                                    stage[32 * oct_:32 * (oct_ + 1), :],
                                    outp[:, :])
                    nc.sync.dma_start(
                        vol_v[:, :, :, base:base + T],
                        stage[:, :].rearrange(
                            "p (dp w) -> p dp w", dp=8)[:, :, :T])
    return nc


def _vertical_rows():
    h = np.arange(H)
    iy = h * (H / (H - 1)) - 0.5
    y0 = np.floor(iy).astype(int)
    wy1 = (iy - y0).astype(np.float32)
    return y0, wy1


def prep_core_inputs(refimg_fea, targetimg_fea, disps, core):
    b = core // 4
    h0 = HS * (core % 4)
    y0, wy1 = _vertical_rows()
    out = {}
    out["dispst"] = np.ascontiguousarray(
        disps[b, :, h0:h0 + HS, :].transpose(1, 0, 2)).astype(np.float32)
    w = np.arange(W, dtype=np.float32)
    out["wrow"] = np.broadcast_to(w * XSCALE - 0.5, (HS, W)).copy()
    tgt = targetimg_fea[b]
    for k in range(NT):
        wp = TOFF[k] + np.arange(128)
        wvalid = (wp >= 0) & (wp < W)
        ga = np.zeros((128, HS, C), np.float32)
        gb = np.zeros((128, HS, C), np.float32)
        tgt_t = np.ascontiguousarray(tgt.transpose(2, 1, 0))  # [W, H, C]
        gh = h0 + np.arange(HS)
        ra, rb = y0[gh], y0[gh] + 1
        rava = (ra >= 0) & (ra < H)
        rbva = (rb >= 0) & (rb < H)
        ga[np.ix_(wvalid, rava)] = tgt_t[wp[wvalid]][:, ra[rava], :]
        gb[np.ix_(wvalid, rbva)] = tgt_t[wp[wvalid]][:, rb[rbva], :]
        out[f"tga{k}"] = ga
        out[f"tgb{k}"] = gb
    out["wyb"] = np.broadcast_to(wy1[h0:h0 + HS], (128, HS)).copy()
    out["refrep"] = np.tile(refimg_fea[b, :, h0:h0 + HS, :],
                            (4, 1, 1)).astype(np.float32)
    p = np.arange(128, dtype=np.float32)
    out["wpb"] = np.stack([-(TOFF[k] + p) for k in range(NT)],
                          axis=1).astype(np.float32)
    bdpm = np.zeros((128, 64), np.float32)
    bdpm[np.arange(128), 32 + np.arange(128) // 32] = 1.0
    out["bdp2"] = bdpm
    return out


_NC_CACHE = {}


def _get_nc(reps=1):
    key = reps
    if key not in _NC_CACHE:
        _NC_CACHE[key] = build_nc(reps=reps)
    return _NC_CACHE[key]


def run(refimg_fea, targetimg_fea, disps, reps=1):
    from concourse.bass_utils import run_bass_kernel_spmd
    nc = _get_nc(reps=reps)
    in_maps = [prep_core_inputs(refimg_fea, targetimg_fea, disps, core)
               for core in range(NCORES)]
    res = run_bass_kernel_spmd(nc, in_maps, core_ids=list(range(NCORES)))
    full = np.empty((B, D, H, W), np.float32)
    for core in range(NCORES):
        b = core // 4
        h0 = HS * (core % 4)
        full[b, :, h0:h0 + HS, :] = res.results[core]["vol"]
    return full


def kernel(refimg_fea, targetimg_fea, disps):
    refimg_fea = np.asarray(refimg_fea, dtype=np.float32)
    targetimg_fea = np.asarray(targetimg_fea, dtype=np.float32)
    disps = np.asarray(disps, dtype=np.float32)
    return run(refimg_fea, targetimg_fea, disps)
